# revision 1
# baseline (speedup 1.0000x reference)
"""Trainium2 Bass kernel for nn_CPCModel (CPC-style NCE loss).

Strategy (8 NeuronCores, full inputs on every core, no collectives):

The reference's leave-one-out softmax pooling collapses algebraically:
    pooled[i] = (T - e_i * zt_i) / (S - e_i),  e = exp(s), S = sum(e), T = sum(e_j zt_j)
so the [B,B] pooling matrix is never materialized.  The loss needs only
    nce = -mean_i( total[i,i] - logsumexp_j total[i,j] )
with  total[i, j in group g] = Azw_g[i]·pooled_g[j] + Czw[i]·c[j] + delta_g[i]
where Azw_g = zw @ Ww_g, Czw = zw @ Wk_w, delta_g = zw @ (Ww_g_b + Wk_b).

Each core redundantly computes the cheap pooling prep for all 4096 rows
(no collectives) and computes its own 512 rows of the [4096,4096] total
matrix + row-wise sum(exp(total - 44)); the diagonal comes from an
elementwise product.  Host sums 8x[128,4] partial row values.

Dtypes: the big matmuls (U = [Czw;Azw_g] builds and the 512x4096 total)
run fp32r (full-rate, ~19-bit mantissa).  The small prep matmuls (zt, h,
s, broadcasts, delta, diag partition-sums) run bf16 — the fp32r ISA mode
requires 128 output partitions and even N, which those shapes violate.
Host does layout prep only (transposes / stacking of weights + zw/c).
"""

import numpy as np

import concourse.bacc as bacc
import concourse.bass as bass
import concourse.mybir as mybir
import concourse.tile as tile
from concourse.bass_utils import run_bass_kernel_spmd

N_CORES = 8
B = 4096
OWN = B // N_CORES            # 512 rows of `total` per core
G = 2048                      # group size
F32 = mybir.dt.float32
F32R = mybir.dt.float32r
BF16 = mybir.dt.bfloat16
AF = mybir.ActivationFunctionType
ALU = mybir.AluOpType
SHIFT = 44.0


def _r(ap):
    return ap.bitcast(F32R)


def _build_program(static_diag=False):
    nc = bacc.Bacc(
        "TRN2",
        target_bir_lowering=False,
        debug=False,
        num_devices=N_CORES,
    )

    def din(name, shape, dt):
        return nc.dram_tensor(name, shape, dt, kind="ExternalInput").ap()

    zwTb_d = din("zwTb", [128, B], BF16)     # concat(zw_0,zw_1).T in bf16
    zwoT_d = din("zwoT", [128, OWN], F32R)   # own 512 rows of zw, transposed
    zwoTb_d = din("zwoTb", [128, OWN], BF16)
    cT_d = din("cT", [64, B], F32R)          # c.T
    uw0_d = din("UW0", [128, 128], F32R)     # hstack(Wk_w, Ww0_w)
    uw1_d = din("UW1", [128, 128], F32R)     # hstack(Wk_w, Ww1_w)
    uwo_d = din("UWo", [128, 128], F32R)     # hstack(Wk_w, Ww_{g(core)})
    lwT0_d = din("lwT0", [128, 64], BF16)    # lin0_w.T
    lwT1_d = din("lwT1", [128, 64], BF16)    # lin1_w.T
    a1wB_d = din("a1wB", [128, 64], BF16)    # blockdiag(a0_1w.T, a1_1w.T)
    a2wB_d = din("a2wB", [64, 2], BF16)      # blockdiag(a0_2w.T, a1_2w.T)
    b0_d = din("b0", [128, 1], BF16)         # Ww0_b + Wk_b
    b1_d = din("b1", [128, 1], BF16)         # Ww1_b + Wk_b
    bo_d = din("b_own", [128, 1], BF16)      # b_{group(core)}
    sel2_d = din("sel2", [2, 128], BF16)     # [[1]*64+[0]*64, [0]*64+[1]*64]
    ones_d = din("ones", [128, 1], BF16)
    linb2_d = din("linb2", [128, 1], F32)    # [lin0_b ; lin1_b]
    a1b2_d = din("a1b2", [64, 1], F32)       # [a0_1b ; a1_1b]
    v_d = nc.dram_tensor("v", [128, 4], F32, kind="ExternalOutput").ap()

    from contextlib import ExitStack
    with tile.TileContext(nc) as tc, ExitStack() as ctx:
        pers = ctx.enter_context(tc.tile_pool(name="pers", bufs=1))
        scr = ctx.enter_context(tc.tile_pool(name="scr", bufs=2))
        pbig = ctx.enter_context(tc.tile_pool(name="pbig", bufs=2, space="PSUM"))
        psml = ctx.enter_context(tc.tile_pool(name="psml", bufs=3, space="PSUM"))
        ptin = ctx.enter_context(tc.tile_pool(name="ptin", bufs=1, space="PSUM"))

        def load(name, shape, src, dt):
            t = pers.tile(shape, dt, tag=name, name=name)
            nc.sync.dma_start(t[:], src[:])
            return t

        zwTb = load("zwTb", [128, B], zwTb_d, BF16)
        zwoT = load("zwoT", [128, OWN], zwoT_d, F32R)
        zwoTb = load("zwoTb", [128, OWN], zwoTb_d, BF16)
        uw0_s = load("uw0_s", [128, 128], uw0_d, F32R)
        uw1_s = load("uw1_s", [128, 128], uw1_d, F32R)
        uwo_s = load("uwo_s", [128, 128], uwo_d, F32R)
        lwT0 = load("lwT0", [128, 64], lwT0_d, BF16)
        lwT1 = load("lwT1", [128, 64], lwT1_d, BF16)
        lwT = [lwT0, lwT1]
        a1wB = load("a1wB", [128, 64], a1wB_d, BF16)
        a2wB = load("a2wB", [64, 2], a2wB_d, BF16)
        b0_s = load("b0_s", [128, 1], b0_d, BF16)
        b1_s = load("b1_s", [128, 1], b1_d, BF16)
        bo_s = load("bo_s", [128, 1], bo_d, BF16)
        sel2 = load("sel2", [2, 128], sel2_d, BF16)
        ones = load("ones", [128, 1], ones_d, BF16)
        linb2 = load("linb2", [128, 1], linb2_d, F32)
        a1b2 = load("a1b2", [64, 1], a1b2_d, F32)

        # ---------- V [128, 4096]: rows 0:64 = cT (direct), 64:128 = pooledT ----------
        V = pers.tile([128, B], F32R, tag="V")
        nc.sync.dma_start(V[0:64, :], cT_d[:])

        # ---------- U_g = [Czw ; Azw_g] via one stacked-weight matmul each ----------
        U0 = pers.tile([128, OWN], F32R, tag="U0")
        U1 = pers.tile([128, OWN], F32R, tag="U1")
        UOwn = pers.tile([128, OWN], F32R, tag="UOwn")
        for U, uw in [(U0, uw0_s), (U1, uw1_s), (UOwn, uwo_s)]:
            pu = psml.tile([128, 512], F32, tag="ps")
            nc.tensor.matmul(pu[:], uw[:], zwoT[:], start=True, stop=True)
            nc.vector.tensor_copy(U[:], pu[:])

        # ---------- delta bias columns: biasS[:, g*4+ic] = zw_own[ic]·b_g - SHIFT ----------
        biasS = pers.tile([128, 8], F32, tag="biasS")
        for g, bg in enumerate([b0_s, b1_s]):
            for ic in range(4):
                pd = ptin.tile([128, 1], F32, tag="pt")
                nc.tensor.matmul(pd[:], zwoTb[:, ic * 128:(ic + 1) * 128], bg[:],
                                 start=True, stop=True)
                nc.scalar.activation(biasS[:, g * 4 + ic:g * 4 + ic + 1], pd[:],
                                     AF.Copy, bias=-SHIFT)

        # ---------- ztT2 [128, 2048] bf16: zt0T on 0:64, zt1T on 64:128 ----------
        ztT2 = pers.tile([128, G], BF16, tag="ztT2")
        for ch in range(4):
            pz = psml.tile([128, 512], F32, tag="ps")
            sl = slice(ch * 512, (ch + 1) * 512)
            nc.tensor.matmul(pz[0:64, :], lwT[0][:], zwTb[:, sl],
                             start=True, stop=True)
            nc.tensor.matmul(pz[64:128, :], lwT[1][:],
                             zwTb[:, G + ch * 512:G + (ch + 1) * 512],
                             start=True, stop=True)
            # relu(x + bias) on DVE: (psum add linb2) max 0
            nc.vector.tensor_scalar(ztT2[:, sl], pz[:], linb2[:], 0.0,
                                    op0=ALU.add, op1=ALU.max)

        # ---------- hT2 [64, 2048] bf16: tanh(zt @ a1w.T + b), block-diag ----------
        hT2 = pers.tile([64, G], BF16, tag="hT2")
        for ch in range(4):
            ph = psml.tile([128, 512], F32, tag="ps")
            sl = slice(ch * 512, (ch + 1) * 512)
            nc.tensor.matmul(ph[0:64, :], a1wB[:], ztT2[:, sl],
                             start=True, stop=True)
            nc.scalar.activation(hT2[:, sl], ph[0:64, :], AF.Tanh, bias=a1b2[:])

        # ---------- scores -> eT2 [2, 2048] bf16, S2 [2,1] f32 ----------
        eT2 = pers.tile([2, G], BF16, tag="eT2")
        Sacc = pers.tile([2, 4], F32, tag="Sacc")
        for ch in range(4):
            ps_ = psml.tile([128, 512], F32, tag="ps")
            sl = slice(ch * 512, (ch + 1) * 512)
            nc.tensor.matmul(ps_[0:2, :], a2wB[:], hT2[:, sl],
                             start=True, stop=True)
            nc.scalar.activation(eT2[:, sl], ps_[0:2, :], AF.Exp,
                                 accum_out=Sacc[:, ch:ch + 1])
        S2 = pers.tile([2, 1], F32, tag="S2")
        nc.vector.reduce_sum(S2[:], Sacc[:], axis=mybir.AxisListType.X)

        # ---------- betaT2 = 1/(e - S)  (= -1/(S - e)) ----------
        bT2a = pers.tile([2, G], F32, tag="bT2a")
        nc.vector.tensor_scalar(bT2a[:], eT2[:], S2[:], None, op0=ALU.subtract)
        bT2 = pers.tile([2, G], BF16, tag="bT2")
        with nc.allow_low_precision(reason="beta in bf16 for PE outer-product"):
            nc.vector.reciprocal(bT2[:], bT2a[:])

        # ---------- ztw = zt * e_bcast (ttr also accumulates T), then pooled ----------
        ztwT2 = pers.tile([128, G], F32, tag="ztwT2")
        Tacc = pers.tile([128, 4], F32, tag="Tacc")
        for ch in range(4):
            sl = slice(ch * 512, (ch + 1) * 512)
            peb = psml.tile([128, 512], F32, tag="ps")
            nc.tensor.matmul(peb[:], sel2[:], eT2[:, sl], start=True, stop=True)
            nc.vector.tensor_tensor(ztwT2[:, sl], ztT2[:, sl], peb[:],
                                    op=ALU.mult)
            nc.vector.reduce_sum(Tacc[:, ch:ch + 1], ztwT2[:, sl],
                                 axis=mybir.AxisListType.X)
        T2 = pers.tile([128, 1], F32, tag="T2")
        nc.vector.reduce_sum(T2[:], Tacc[:], axis=mybir.AxisListType.X)

        # pooled = (ztw - T) * beta_bcast   (beta = -1/(S-e) so signs cancel)
        pooled2 = pers.tile([128, G], F32, tag="pooled2")
        for ch in range(4):
            sl = slice(ch * 512, (ch + 1) * 512)
            pbb = psml.tile([128, 512], F32, tag="ps")
            nc.tensor.matmul(pbb[:], sel2[:], bT2[:, sl], start=True, stop=True)
            nc.vector.scalar_tensor_tensor(
                out=pooled2[:, sl], in0=ztwT2[:, sl], scalar=T2[:], in1=pbb[:],
                op0=ALU.subtract, op1=ALU.mult)

        # V rows 64:128: group1 pooled at cols 2048:4096 (converting copy),
        # group0 via partition-shifting sbuf->sbuf DMA (bit-identical f32).
        nc.vector.tensor_copy(V[64:128, G:B], pooled2[64:128, :])
        nc.sync.dma_start(V[64:128, 0:G], _r(pooled2[0:64, :]))

        # ---------- main loop: total rows (own 512) x all 4096 cols ----------
        seacc = pers.tile([128, 16], F32, tag="seacc")
        for ic in range(4):
            usl = slice(ic * 128, (ic + 1) * 128)
            for pair in range(4):
                g = pair // 2
                U = U0 if g == 0 else U1
                pm = pbig.tile([128, 1024], F32, tag="pb")
                for half in range(2):
                    jt = pair * 2 + half
                    nc.tensor.matmul(
                        pm[:, half * 512:(half + 1) * 512],
                        U[:, usl],
                        V[:, jt * 512:(jt + 1) * 512],
                        start=True, stop=True)
                es = scr.tile([128, 1024], BF16, tag="escr")
                nc.scalar.activation(
                    es[:], pm[:], AF.Exp,
                    bias=biasS[:, g * 4 + ic:g * 4 + ic + 1],
                    accum_out=seacc[:, ic * 4 + pair:ic * 4 + pair + 1])

        seall = pers.tile([128, 4], F32, tag="seall")
        for ic in range(4):
            nc.vector.reduce_sum(seall[:, ic:ic + 1], seacc[:, ic * 4:(ic + 1) * 4],
                                 axis=mybir.AxisListType.X)
        lnall = pers.tile([128, 4], F32, tag="lnall")
        nc.scalar.activation(lnall[:], seall[:], AF.Ln)

        # ---------- diagonal: diag[i] = UOwn[:,i]·V[:,own_pos(i)] ----------
        if static_diag:
            vsl = slice(0, OWN)
        else:
            pid = nc.vector.partition_id()
            vsl = bass.ts(pid, OWN)
        prod2 = pers.tile([128, OWN], BF16, tag="prod2")
        nc.vector.tensor_tensor(prod2[:], UOwn[:].bitcast(F32),
                                V[:, vsl].bitcast(F32), op=ALU.mult)

        vall = pers.tile([128, 4], F32, tag="vall")
        for ic in range(4):
            pdg = ptin.tile([128, 1], F32, tag="pt")
            nc.tensor.matmul(pdg[:], prod2[:, ic * 128:(ic + 1) * 128], ones[:],
                             start=True, stop=False)
            nc.tensor.matmul(pdg[:], zwoTb[:, ic * 128:(ic + 1) * 128], bo_s[:],
                             start=False, stop=True)
            # v = (diag_raw + delta - 44) - ln(sumexp)
            nc.vector.scalar_tensor_tensor(
                out=vall[:, ic:ic + 1], in0=pdg[:], scalar=-SHIFT,
                in1=lnall[:, ic:ic + 1], op0=ALU.add, op1=ALU.subtract)

        nc.sync.dma_start(v_d[:], vall[:])

    nc.compile()
    return nc


_built = None


def _get_program():
    global _built
    if _built is None:
        _built = _build_program()
    return _built


def make_in_maps(inputs):
    import ml_dtypes
    BF = ml_dtypes.bfloat16
    f = lambda x: np.ascontiguousarray(np.asarray(x, dtype=np.float32))
    bf = lambda x: np.ascontiguousarray(np.asarray(x, np.float32).astype(BF))

    zw = np.concatenate([f(inputs['zw_0']), f(inputs['zw_1'])], axis=0)
    zwT = np.ascontiguousarray(zw.T)
    b0 = f(inputs['Ww0_b']) + f(inputs['Wk_b'])
    b1 = f(inputs['Ww1_b']) + f(inputs['Wk_b'])

    a1wB = np.zeros((128, 64), np.float32)
    a1wB[0:64, 0:32] = f(inputs['a0_1w']).T
    a1wB[64:128, 32:64] = f(inputs['a1_1w']).T
    a2wB = np.zeros((64, 2), np.float32)
    a2wB[0:32, 0:1] = f(inputs['a0_2w']).T
    a2wB[32:64, 1:2] = f(inputs['a1_2w']).T
    sel2 = np.zeros((2, 128), np.float32)
    sel2[0, 0:64] = 1.0
    sel2[1, 64:128] = 1.0
    linb2 = np.concatenate([f(inputs['lin0_b']), f(inputs['lin1_b'])])
    a1b2 = np.concatenate([f(inputs['a0_1b']), f(inputs['a1_1b'])])
    wk = f(inputs['Wk_w'])
    uw0 = np.hstack([wk, f(inputs['Ww0_w'])])   # [128,128]
    uw1 = np.hstack([wk, f(inputs['Ww1_w'])])

    base = {
        'zwTb': bf(zwT),
        'cT': np.ascontiguousarray(f(inputs['c']).T),
        'UW0': uw0,
        'UW1': uw1,
        'lwT0': bf(f(inputs['lin0_w']).T),
        'lwT1': bf(f(inputs['lin1_w']).T),
        'a1wB': bf(a1wB),
        'a2wB': bf(a2wB),
        'b0': bf(b0.reshape(128, 1)),
        'b1': bf(b1.reshape(128, 1)),
        'sel2': bf(sel2),
        'ones': bf(np.ones((128, 1), np.float32)),
        'linb2': linb2.reshape(128, 1),
        'a1b2': a1b2.reshape(64, 1),
    }
    in_maps = []
    for cid in range(N_CORES):
        g = cid // 4
        m = dict(base)
        zo = np.ascontiguousarray(zwT[:, cid * OWN:(cid + 1) * OWN])
        m['zwoT'] = zo
        m['zwoTb'] = bf(zo)
        m['UWo'] = uw0 if g == 0 else uw1
        m['b_own'] = bf((b0 if g == 0 else b1).reshape(128, 1))
        in_maps.append(m)
    return in_maps


def kernel(**inputs):
    nc = _get_program()
    in_maps = make_in_maps(inputs)
    res = run_bass_kernel_spmd(nc, in_maps, list(range(N_CORES)))
    tot = 0.0
    for r in res.results:
        tot += np.asarray(r['v'], dtype=np.float64).sum()
    return np.array(-(tot / B), dtype=np.float32)



# revision 15
# speedup vs baseline: 1.4990x; 1.4990x over previous
"""Trainium2 Bass kernel for nn_CPCModel (CPC-style NCE loss) — v2.

Math (identical to v1): the leave-one-out softmax pooling collapses to
    pooled_j = (T - e_j zt_j) / (S - e_j),  e = exp(s), S = sum e, T = sum e_j zt_j
and the loss needs only  -mean_i(t_ii - lse_i)  with
    t_ij = C_i.c_j + A_g(j),i . pooled_j + delta_g(j),i
    C = Wk^T zw,  A_g = Ww_g^T zw,  delta_g = zw . (Ww_g_b + Wk_b).

v2 structural changes vs v1:
  * No V assembly: each 512-col psum bank of a total tile accumulates TWO
    matmuls — a K=65 c-part (cTx carries a ones row so delta rides the
    contraction; the exp bias is the constant -44) and a K=64 pooled-part
    reading pooled2 slices directly.  Tiles are mixed-group [128, 2048]
    with one accum column each.
  * ln(sumexp), delta bookkeeping and the final reduction run on the HOST
    (kernel outputs raw exp row-sums + raw diag), so a single act table
    (exp_and_others: tanh+exp+identity) covers the whole kernel.
  * Inputs arrive as 5 packed DMAs in dependency order; the v1
    pooled->V partition-shift DMA is gone.
  * PE warm-up matmuls reach the full 2.4GHz p-state before the first
    real matmul; ztw+T fuse into tensor_tensor_reduce; (e-S) runs on the
    Act engine (Identity+bias); U/diag psum->sbuf copies run on GpSimd;
    one pooled chunk runs on GpSimd.
  * All matmuls are bf16 (f32 only in psum/accum paths).
"""

import numpy as np

import concourse.bacc as bacc
import concourse.bass as bass
import concourse.mybir as mybir
import concourse.tile as tile
from concourse.bass_utils import run_bass_kernel_spmd

N_CORES = 8
B = 4096
OWN = 512                     # rows of `total` per core
G = 2048                      # group size
CW = 1024                     # prep chunk width (cols per group per chunk)
F32 = mybir.dt.float32
BF16 = mybir.dt.bfloat16
AF = mybir.ActivationFunctionType
ALU = mybir.AluOpType
SHIFT = 44.0
N_WARMUP = 14                 # PE p-state warm-up matmuls
# main-loop tiles (ic, q): q<2 need pooled chunk 0 only, q>=2 chunk 1
TILES = [(0, 0), (1, 0), (0, 1), (1, 1), (2, 0), (3, 0), (2, 1), (3, 1),
         (0, 2), (1, 2), (0, 3), (1, 3), (2, 2), (3, 2), (2, 3), (3, 3)]

# wbf column layout (shared bf16 weight pack)
WB_LWT0 = 0       # [128, 64]  lin0_w.T
WB_LWT1 = 64      # [128, 64]  lin1_w.T
WB_A1WB = 128     # [128, 64]  blockdiag(a0_1w.T, a1_1w.T)
WB_A2P = 192      # [128, 4]   rows c*64+g*32+k -> col 2c+g = a_g2w[k]
WB_ONES = 196     # [128, 1]   ones
WB_SEL4 = 197     # [34, 128]  sel4[c*32+g, 64g:64(g+1)] = 1
WB_F32 = 326      # [128, 12]  bf16-pair view of [128, 6] f32: linb2, a1b4, MselN
WB_END = 338

# per-core bf16 pack: host-precomputed own-row coefficient matrices
PC_UA = 0         # [128, 512] rows g*64+f = A_g-coords of own rows
PC_UC0 = 512      # [65, 512]  rows 0:64 = C-coords, row 64 = delta_0
PC_UC1 = 1024     # [65, 512]
PC_UD = 1536      # [128, 512] rows g*64.. = A_g-own (own group), rest 0
PC_UCO = 2048     # [65, 512]  UC of own group
PC_END = 2560


def _build_program(static_diag=False, use_ttr=False, use_actbias=True):
    nc = bacc.Bacc(
        "TRN2",
        target_bir_lowering=False,
        debug=False,
        num_devices=N_CORES,
    )

    def din(name, shape, dt):
        return nc.dram_tensor(name, shape, dt, kind="ExternalInput").ap()

    wbf_d = din("wbf", [128, WB_END], BF16)
    zwT_d = din("zwT", [128, B], BF16)     # cols: g0 j0..2047 | g1 j0..2047
    cTx_d = din("cTx", [65, B], BF16)      # rows 0:64 = c^T (global cols), row 64 = 1
    pcb_d = din("pcb", [128, PC_END], BF16)
    vse_d = nc.dram_tensor("v_se", [128, 16], F32, kind="ExternalOutput").ap()
    vdg_d = nc.dram_tensor("v_diag", [1, OWN], F32, kind="ExternalOutput").ap()

    from contextlib import ExitStack
    with tile.TileContext(nc) as tc, ExitStack() as ctx:
        pers = ctx.enter_context(tc.tile_pool(name="pers", bufs=1))
        scr = ctx.enter_context(tc.tile_pool(name="scr", bufs=2))
        pbig = ctx.enter_context(tc.tile_pool(name="pbig", bufs=4, space="PSUM"))

        # ---- persistent SBUF tiles -------------------------------------
        wbf = pers.tile([128, WB_END], BF16, tag="wbf", name="wbf")
        zwT = pers.tile([128, B], BF16, tag="zwT", name="zwT")
        cTx = pers.tile([65, B], BF16, tag="cTx", name="cTx")
        pcb = pers.tile([128, PC_END], BF16, tag="pcb", name="pcb")
        wz = pers.tile([128, 256], BF16, tag="wz", name="wz")
        onesW = pers.tile([128, 1], BF16, tag="onesW", name="onesW")
        ztT2 = pers.tile([128, G], BF16, tag="ztT2", name="ztT2")
        hT = pers.tile([128, CW], BF16, tag="hT", name="hT")
        e4 = pers.tile([34, CW], BF16, tag="e4", name="e4")
        Sacc = pers.tile([34, 1], F32, tag="Sacc", name="Sacc")
        S4sb = pers.tile([34, 1], F32, tag="S4sb", name="S4sb")
        rsub4 = pers.tile([34, CW], F32, tag="rsub4", name="rsub4")
        beta4 = pers.tile([34, CW], BF16, tag="beta4", name="beta4")
        ztwT = pers.tile([128, G], BF16, tag="ztwT", name="ztwT")
        Tacc = pers.tile([128, 2], F32, tag="Tacc", name="Tacc")
        T2 = pers.tile([128, 1], F32, tag="T2", name="T2")
        pooled2 = pers.tile([128, G], BF16, tag="pooled2", name="pooled2")
        prodA = pers.tile([128, OWN], BF16, tag="prodA", name="prodA")
        prodC = pers.tile([65, OWN], BF16, tag="prodC", name="prodC")
        seacc = pers.tile([128, 16], F32, tag="seacc", name="seacc")
        diagv = pers.tile([1, OWN], F32, tag="diagv", name="diagv")
        bias44 = pers.tile([128, 1], F32, tag="bias44", name="bias44")

        # ---- PSUM: one rotating pool, 4 x [128, 1024] generations ----
        def m():
            return pbig.tile([128, 1024], F32, tag="m", name="pm")

        # ---- input DMAs (dependency order) -----------------------------
        nc.sync.dma_start(wbf[:], wbf_d[:])
        nc.sync.dma_start(zwT[:, 0:2 * CW], zwT_d[:, 0:2 * CW])
        nc.sync.dma_start(zwT[:, 2 * CW:B], zwT_d[:, 2 * CW:B])
        nc.sync.dma_start(cTx[:], cTx_d[:])
        nc.sync.dma_start(pcb[:], pcb_d[:])

        UAsb = pcb[:, PC_UA:PC_UA + OWN]
        UC0sb = pcb[0:65, PC_UC0:PC_UC0 + OWN]
        UC1sb = pcb[0:65, PC_UC1:PC_UC1 + OWN]
        UDsb = pcb[:, PC_UD:PC_UD + OWN]
        UCosb = pcb[0:65, PC_UCO:PC_UCO + OWN]
        wf32 = wbf[:, WB_F32:WB_F32 + 12].bitcast(F32)
        linb2 = wf32[:, 0:1]
        a1b4 = wf32[:, 1:2]
        MselN = wf32[0:34, 2:4]
        lwT = [wbf[:, WB_LWT0:WB_LWT0 + 64], wbf[:, WB_LWT1:WB_LWT1 + 64]]
        a1wB = wbf[:, WB_A1WB:WB_A1WB + 64]
        ones128 = wbf[:, WB_ONES:WB_ONES + 1]
        sel4 = wbf[0:34, WB_SEL4:WB_SEL4 + 128]

        nc.gpsimd.memset(onesW[:], 1.0)
        nc.gpsimd.memset(wz[:], 0.0)
        nc.gpsimd.memset(e4[:], 1.0)
        nc.gpsimd.memset(Sacc[:], 0.0)
        nc.gpsimd.memset(S4sb[:], 0.0)
        nc.gpsimd.memset(bias44[:], -SHIFT)

        # ---- PE warm-up ------------------------------------------------
        warm = m()
        for _ in range(N_WARMUP):
            nc.tensor.matmul(warm[0:1, 0:256], onesW[:], wz[:],
                             start=True, stop=True)

        # zwT is packed chunk-major: col (c*2048 + g*1024 + w) holds group
        # g's column c*CW + w.  Chunk c covers cols c*CW..(c+1)*CW of EACH
        # group (fused rows: g0 -> partitions 0:64, g1 -> 64:128).

        # ---- zt = relu(zw @ lw.T + b) ----------------------------------
        pz = []
        for c in range(2):
            pz.append(m())
            for g in range(2):
                for hf in range(2):
                    nc.tensor.matmul(
                        pz[c][g * 64:(g + 1) * 64, hf * 512:(hf + 1) * 512],
                        lwT[g],
                        zwT[:, c * 2 * CW + g * CW + hf * 512:
                            c * 2 * CW + g * CW + (hf + 1) * 512],
                        start=True, stop=True)
            nc.vector.tensor_scalar(ztT2[:, c * CW:(c + 1) * CW], pz[c][:],
                                    linb2, 0.0, op0=ALU.add, op1=ALU.max)

        # ---- h/tanh: hT [128, CW], chunk c on partitions c*64.. --------
        for c in range(2):
            ph = m()
            for hf in range(2):
                nc.tensor.matmul(
                    ph[0:64, hf * 512:(hf + 1) * 512],
                    a1wB,
                    ztT2[:, c * CW + hf * 512:c * CW + (hf + 1) * 512],
                    start=True, stop=True)
            nc.scalar.activation(hT[c * 64:(c + 1) * 64, :], ph[0:64, :],
                                 AF.Tanh, bias=a1b4[c * 64:(c + 1) * 64, :])

        # ---- scores chunk 0 --------------------------------------------
        pe4t = m()
        for hf in range(2):
            nc.tensor.matmul(
                pe4t[0:2, hf * 512:(hf + 1) * 512],
                wbf[0:64, WB_A2P:WB_A2P + 2],
                hT[0:64, hf * 512:(hf + 1) * 512],
                start=True, stop=True)
        nc.scalar.activation(e4[0:2, :], pe4t[0:2, :],
                             AF.Exp, accum_out=Sacc[0:2, :])

        # ---- scores chunk 1 (own psum generation) ----------------------
        pe4t2 = m()
        for hf in range(2):
            nc.tensor.matmul(
                pe4t2[32:34, hf * 512:(hf + 1) * 512],
                wbf[64:128, WB_A2P + 2:WB_A2P + 4],
                hT[64:128, hf * 512:(hf + 1) * 512],
                start=True, stop=True)
        nc.scalar.activation(e4[32:34, :], pe4t2[32:34, :],
                             AF.Exp, accum_out=Sacc[32:34, :])

        # ---- S4n = -S_group(row), tiny mms into pe4t2's spare column ---
        nc.tensor.matmul(pe4t2[0:2, 1023:1024], MselN, Sacc[:],
                         start=True, stop=True)
        nc.tensor.matmul(pe4t2[32:34, 1023:1024], MselN, Sacc[:],
                         start=True, stop=True)

        # ---- ztw = zt * e_bcast (TTR also accumulates T) ---------------
        for c in range(2):
            peb = m()
            for hf in range(2):
                nc.tensor.matmul(
                    peb[:, hf * 512:(hf + 1) * 512],
                    sel4[c * 32:c * 32 + 2, :],
                    e4[c * 32:c * 32 + 2, hf * 512:(hf + 1) * 512],
                    start=True, stop=True)
            if use_ttr:
                with nc.allow_low_precision(reason="ztw bf16; accum f32"):
                    nc.vector.tensor_tensor_reduce(
                        out=ztwT[:, c * CW:(c + 1) * CW],
                        in0=ztT2[:, c * CW:(c + 1) * CW],
                        in1=peb[:],
                        scale=1.0, scalar=0.0,
                        op0=ALU.mult, op1=ALU.add,
                        accum_out=Tacc[:, c:c + 1])
            else:
                with nc.allow_low_precision(reason="ztw bf16; accum f32"):
                    nc.vector.scalar_tensor_tensor(
                        out=ztwT[:, c * CW:(c + 1) * CW],
                        in0=ztT2[:, c * CW:(c + 1) * CW],
                        scalar=0.0, in1=peb[:],
                        op0=ALU.add, op1=ALU.mult,
                        accum_out=Tacc[:, c:c + 1])
            if c == 0:
                nc.vector.tensor_copy(S4sb[0:2, :], pe4t2[0:2, 1023:1024])
                nc.vector.tensor_copy(S4sb[32:34, :], pe4t2[32:34, 1023:1024])
                # rsub = e - S (Identity, bias = -S), col-halves so the
                # reciprocal + pooled chain pipelines at 512 granularity
                for hh in range(2):
                    if use_actbias:
                        nc.scalar.activation(
                            rsub4[:, hh * 512:(hh + 1) * 512],
                            e4[:, hh * 512:(hh + 1) * 512],
                            AF.Identity, bias=S4sb[:])
                    else:
                        nc.vector.tensor_scalar(
                            rsub4[:, hh * 512:(hh + 1) * 512],
                            e4[:, hh * 512:(hh + 1) * 512],
                            S4sb[:], None, op0=ALU.add)
        with nc.allow_low_precision(reason="beta bf16 for bcast matmul"):
            for hh in range(2):
                nc.vector.reciprocal(beta4[:, hh * 512:(hh + 1) * 512],
                                     rsub4[:, hh * 512:(hh + 1) * 512])
        nc.vector.tensor_tensor(T2[:], Tacc[:, 0:1], Tacc[:, 1:2],
                                op=ALU.add)

        # ---- pooled = (ztw - T) * beta_bcast ---------------------------
        pbb = []
        for c in range(2):
            pbb.append(m())
            for hf in range(2):
                nc.tensor.matmul(
                    pbb[c][:, hf * 512:(hf + 1) * 512],
                    sel4[c * 32:c * 32 + 2, :],
                    beta4[c * 32:c * 32 + 2, hf * 512:(hf + 1) * 512],
                    start=True, stop=True)
        with nc.allow_low_precision(reason="pooled bf16 for PE rhs"):
            for hh in range(2):
                sl = slice(hh * 512, (hh + 1) * 512)
                nc.vector.scalar_tensor_tensor(
                    out=pooled2[:, sl], in0=ztwT[:, sl], scalar=T2[:],
                    in1=pbb[0][:, sl], op0=ALU.subtract, op1=ALU.mult)
                sl1 = slice(CW + hh * 512, CW + (hh + 1) * 512)
                nc.vector.scalar_tensor_tensor(
                    out=pooled2[:, sl1], in0=ztwT[:, sl1], scalar=T2[:],
                    in1=pbb[1][:, hh * 512:(hh + 1) * 512],
                    op0=ALU.subtract, op1=ALU.mult)

        # ---- diag products ---------------------------------------------
        if static_diag:
            own_global = slice(0, OWN)
            own_ingrp = slice(0, OWN)
        else:
            pid = nc.vector.partition_id()
            own_global = bass.ds(pid * OWN, OWN)       # cTx global cols
            own_ingrp = bass.ds((pid % 4) * OWN, OWN)  # pooled2 cols
        with nc.allow_low_precision(reason="diag prods bf16"):
            nc.vector.tensor_tensor(prodC[:], UCosb[:], cTx[:, own_global],
                                    op=ALU.mult)

        # ---- main loop: 16 tiles [128, 1024] ----------------------------
        # tile (ic, q): bank b = group, cols = g*G + q*512
        UCsb = [UC0sb, UC1sb]
        for ti, (ic, q) in enumerate(TILES):
            pt = m()
            isl = slice(ic * 128, (ic + 1) * 128)
            for g in range(2):
                j0 = q * 512
                out = pt[:, g * 512:(g + 1) * 512]
                nc.tensor.matmul(out, UCsb[g][:, isl],
                                 cTx[:, g * G + j0:g * G + j0 + 512],
                                 start=True, stop=False)
            for g in range(2):
                j0 = q * 512
                out = pt[:, g * 512:(g + 1) * 512]
                nc.tensor.matmul(out, UAsb[g * 64:(g + 1) * 64, isl],
                                 pooled2[g * 64:(g + 1) * 64, j0:j0 + 512],
                                 start=False, stop=True)
            es = scr.tile([128, 1024], BF16, tag="es", name="es")
            nc.scalar.activation(es[:], pt[:], AF.Exp, bias=bias44[:],
                                 accum_out=seacc[:, ti:ti + 1])

            if ti == 1:
                with nc.allow_low_precision(reason="diag prods bf16"):
                    nc.vector.tensor_tensor(prodA[:], UDsb[:],
                                            pooled2[:, own_ingrp],
                                            op=ALU.mult)
            if ti == 3:
                pdg = m()
                nc.tensor.matmul(pdg[0:1, 0:OWN], ones128[0:65, :], prodC[:],
                                 start=True, stop=False)
                nc.tensor.matmul(pdg[0:1, 0:OWN], ones128, prodA[:],
                                 start=False, stop=True)
                nc.vector.tensor_copy(diagv[:], pdg[0:1, 0:OWN])
                nc.sync.dma_start(vdg_d[:], diagv[:])

        nc.sync.dma_start(vse_d[:, 0:14], seacc[:, 0:14])
        nc.sync.dma_start(vse_d[:, 14:16], seacc[:, 14:16])

    nc.compile()
    return nc


_built = None


def _get_program():
    global _built
    if _built is None:
        _built = _build_program()
    return _built


def make_in_maps(inputs):
    import ml_dtypes
    BF = ml_dtypes.bfloat16
    f = lambda x: np.ascontiguousarray(np.asarray(x, dtype=np.float32))
    bf = lambda x: np.ascontiguousarray(np.asarray(x, np.float32).astype(BF))

    zw0, zw1 = f(inputs['zw_0']), f(inputs['zw_1'])
    zw = np.concatenate([zw0, zw1], axis=0)          # [4096, 128]
    zwT = np.ascontiguousarray(zw.T)                 # [128, 4096]
    wk = f(inputs['Wk_w'])                           # [128, 64]
    b0 = f(inputs['Ww0_b']) + f(inputs['Wk_b'])      # [128]
    b1 = f(inputs['Ww1_b']) + f(inputs['Wk_b'])
    bg = [b0, b1]
    ww = [f(inputs['Ww0_w']), f(inputs['Ww1_w'])]    # [128, 64] each

    wbf = np.zeros((128, WB_END), np.float32)
    wbf[:, WB_LWT0:WB_LWT0 + 64] = f(inputs['lin0_w']).T
    wbf[:, WB_LWT1:WB_LWT1 + 64] = f(inputs['lin1_w']).T
    wbf[0:64, WB_A1WB:WB_A1WB + 32] = f(inputs['a0_1w']).T
    wbf[64:128, WB_A1WB + 32:WB_A1WB + 64] = f(inputs['a1_1w']).T
    # a2P: rows c*64 + g*32 + k, col 2c+g
    for c in range(2):
        for g in range(2):
            a2 = f(inputs[f'a{g}_2w'])[0]            # [32]
            wbf[c * 64 + g * 32:c * 64 + (g + 1) * 32, WB_A2P + 2 * c + g] = a2
    wbf[:, WB_ONES] = 1.0
    for c in range(2):
        for g in range(2):
            wbf[c * 32 + g, WB_SEL4 + 64 * g:WB_SEL4 + 64 * (g + 1)] = 1.0

    wf32 = np.zeros((128, 6), np.float32)
    wf32[:, 0] = np.concatenate([f(inputs['lin0_b']), f(inputs['lin1_b'])])
    a1b2 = np.concatenate([f(inputs['a0_1b']), f(inputs['a1_1b'])])  # [64]
    wf32[:, 1] = np.concatenate([a1b2, a1b2])
    # MselN [34, 2]: col g sums (negated) the accum rows of group g; only
    # rows {0, 1, 32, 33} ever hold nonzero accums (rest memset to 0).
    for p in range(34):
        for r in range(2):
            if (p % 2) == r:
                wf32[p, 2 + r] = -1.0

    cTx = np.zeros((65, B), np.float32)
    cTx[0:64, :] = f(inputs['c']).T
    cTx[64, :] = 1.0

    # chunk-major zw pack: col (c*2048 + g*1024 + w) = group g, col c*1024+w
    zwTp = np.zeros((128, B), np.float32)
    for c in range(2):
        for g in range(2):
            zwTp[:, c * 2048 + g * 1024:c * 2048 + (g + 1) * 1024] = \
                zwT[:, g * 2048 + c * 1024:g * 2048 + (c + 1) * 1024]
    wbf16 = np.asarray(wbf, np.float32).astype(BF)
    pair = wf32.astype(np.float32).view(np.uint16).reshape(128, 12)
    full = np.zeros((128, WB_END), BF)
    full[:, 0:WB_END - 12] = wbf16[:, 0:WB_END - 12]
    full[:, WB_F32:WB_F32 + 12] = pair.view(BF)
    base = {
        'wbf': full,
        'zwT': bf(zwTp),
        'cTx': bf(cTx),
    }
    in_maps = []
    for cid in range(N_CORES):
        g = cid // 4
        zwo = zw[cid * OWN:(cid + 1) * OWN, :]        # [512, 128]
        A = [zwo @ ww[0], zwo @ ww[1]]                 # [512, 64] each
        C = zwo @ wk                                   # [512, 64]
        pcb = np.zeros((128, PC_END), np.float32)
        pcb[0:64, PC_UA:PC_UA + OWN] = A[0].T
        pcb[64:128, PC_UA:PC_UA + OWN] = A[1].T
        for gg, base_ in ((0, PC_UC0), (1, PC_UC1)):
            pcb[0:64, base_:base_ + OWN] = C.T
            pcb[64, base_:base_ + OWN] = zwo @ bg[gg]
        pcb[g * 64:(g + 1) * 64, PC_UD:PC_UD + OWN] = A[g].T
        pcb[0:64, PC_UCO:PC_UCO + OWN] = C.T
        pcb[64, PC_UCO:PC_UCO + OWN] = zwo @ bg[g]
        m = dict(base)
        m['pcb'] = bf(pcb)
        in_maps.append(m)
    return in_maps


def kernel(**inputs):
    nc = _get_program()
    in_maps = make_in_maps(inputs)
    res = run_bass_kernel_spmd(nc, in_maps, list(range(N_CORES)))
    # host epilogue: lse_i = SHIFT + ln(sum_t seacc), nce = -mean(diag - lse)
    total = 0.0
    for cid, r in enumerate(res.results):
        se = np.asarray(r['v_se'], dtype=np.float64)      # [128, 16]
        dg = np.asarray(r['v_diag'], dtype=np.float64)[0]  # [512]
        se_rows = np.zeros((4, 128), np.float64)
        for ti, (ic, q) in enumerate(TILES):
            se_rows[ic] += se[:, ti]
        lse = SHIFT + np.log(se_rows.reshape(512))         # i_local = ic*128+p
        total += np.sum(dg - lse)
    return np.array(-(total / B), dtype=np.float32)
